# revision 28
# baseline (speedup 1.0000x reference)
"""Trainium2 Bass kernel for nn_Attention_23364622090354.

Attention with RoPE + flat QK-RMSNorm + GQA (16 q heads, 4 kv heads) +
causal softmax. B=2, S=2048, DIM=2048, HD=128.

Sharding (8 NeuronCores = 2 batches x 4-way head tensor-parallel):
  core c -> batch b = c//4, head group g = c%4 (q heads 4g..4g+3, kv head g).
Every core runs the identical causal program. Collectives: per-chunk
AllReduce of sum-of-squares rows (RMSNorm over flattened heads), and a
per-head 8-core AllToAll of the attention output so each core runs the
output projection for its own 512 sequence rows.

This version interleaves the projection chunks with the attention rounds
(m-outer) so the PE never drains at a phase boundary, restricts causal
diagonal blocks to their valid columns, computes every reciprocal as
exp(-ln x) on the scalar engine (single activation table), does rope with
3 full-width vector ops + a DMA partition-swap, and spreads epilogue
element-wise work across vector/scalar/gpsimd.
"""
import copy

import numpy as np
import ml_dtypes

import concourse.bass as bass
import concourse.mybir as mybir
from concourse.tile import TileContext
from concourse.vector_clock import ScopedClock
from concourse import tile as _tile_mod

BF = ml_dtypes.bfloat16
F32, BF16 = mybir.dt.float32, mybir.dt.bfloat16

B, S, DIM = 2, 2048, 2048
NH, NKV, HD = 16, 4, 128
TP = 4
HPC = NH // TP            # q heads per core = 4
EPS = 1e-6
SCALE = float(HD) ** (-0.5)
NT = S // 128             # 16 token tiles
ND = DIM // 128           # 16 contraction tiles
TOK = S // TP             # 512 tokens owned per core after A2A

AluOp = mybir.AluOpType
AFT = mybir.ActivationFunctionType


# ---------------------------------------------------------------- patches --
_ws_counter = [0]


def _split_sync_waits(nc, limit=1):
    """This neuronxcc rejects >1 sem wait per instruction; move extras onto
    same-engine NoOps placed immediately before (engines run in order)."""
    tmpl = nc.sync.nop(nofuse=True, hint="waitsplit-template").ins
    for fn in nc.m.functions:
        for bb in fn.blocks:
            if tmpl in bb.instructions:
                bb.instructions.remove(tmpl)
    for fn in nc.m.functions:
        for bb in fn.blocks:
            out = []
            changed = False
            for inst in bb.instructions:
                si = inst.sync_info
                waits = list(si.on_wait) if si is not None and si.on_wait else []
                if len(waits) > limit:
                    for w in waits[:-limit]:
                        _ws_counter[0] += 1
                        nop = copy.copy(tmpl)
                        nop.name = f"I-waitsplit-{_ws_counter[0]}"
                        nop.engine = inst.engine
                        nop.sync_info = mybir.SyncInfo(on_wait=[w], on_update=[])
                        out.append(nop)
                    si.on_wait = waits[-limit:]
                    changed = True
                out.append(inst)
            if changed:
                try:
                    bb.instructions[:] = out
                except TypeError:
                    bb.instructions = out


def _patched_drain_and_barrier(self, tick_clock, wait_clock):
    """Kernel-tail drain with waits redistributed to 1-wait NOPs."""
    nc = self.nc
    probe = nc.sync.nop(nofuse=True, hint="drain_waits")
    wait_clock.add_sem_waits(probe.ins, ScopedClock({None: tick_clock.global_clock}))
    si = probe.ins.sync_info
    waits = list(si.on_wait or []) if si is not None else []
    if len(waits) > 1:
        si.on_wait = waits[:1]
        for w in waits[1:]:
            extra = nc.sync.nop(nofuse=True, hint="drain_waits")
            extra.ins.sync_info = mybir.SyncInfo(on_wait=[w], on_update=[])
    nc.sync.drain()
    nc.all_engine_barrier()
    assert self.sems is not None
    popped = nc._tile_sem_poison_stack.pop()
    assert popped is self._sem_poison
    nc.clear_and_free_semaphores(list(self.sems.allocated().values()))
    nc.all_engine_barrier()


_tile_mod.TileContext._drain_and_barrier = _patched_drain_and_barrier


# ------------------------------------------------------------------ graph --
def build_graph(debug=False):
    nc = bass.Bass()
    xt_d = nc.dram_tensor("xt", [DIM, S], BF16, kind="ExternalInput")
    wqt_d = nc.dram_tensor("wqt", [DIM, HPC * HD], BF16, kind="ExternalInput")
    wkt_d = nc.dram_tensor("wkt", [DIM, HD], BF16, kind="ExternalInput")
    wvt_d = nc.dram_tensor("wvt", [DIM, HD], BF16, kind="ExternalInput")
    wot_d = nc.dram_tensor("wot", [NH * HD, DIM], BF16, kind="ExternalInput")
    f12_d = nc.dram_tensor("f12", [128, 2, S], BF16, kind="ExternalInput")
    wcol_d = nc.dram_tensor("wcol", [HD, HPC], F32, kind="ExternalInput")
    masks_d = nc.dram_tensor("masks", [128, 512], BF16, kind="ExternalInput")
    bsel_d = nc.dram_tensor("bsel", [128, 2], F32, kind="ExternalInput")
    out_d = nc.dram_tensor("out", [TOK, DIM], F32, kind="ExternalOutput")
    dbg = {}
    if debug:
        dbg["qt"] = nc.dram_tensor("dbg_qt", [128, HPC, S], BF16, kind="ExternalOutput")
        dbg["kt"] = nc.dram_tensor("dbg_kt", [128, S], BF16, kind="ExternalOutput")
        dbg["v"] = nc.dram_tensor("dbg_v", [128, NT, HD], BF16, kind="ExternalOutput")
        dbg["ssq"] = nc.dram_tensor("dbg_ssq", [2, S], F32, kind="ExternalOutput")
        dbg["at"] = nc.dram_tensor("dbg_at", [128, NH, 512], BF16,
                                   kind="ExternalOutput")

    groups4 = [[0, 1, 2, 3], [4, 5, 6, 7]]
    groups8 = [list(range(8))]

    from contextlib import ExitStack
    with TileContext(nc) as tc, ExitStack() as outer:
        consts = outer.enter_context(tc.tile_pool(name="consts", bufs=1))
        dram = outer.enter_context(tc.tile_pool(name="dram", bufs=1, space="DRAM"))

        f12_sb = consts.tile([128, 2, S], BF16)
        masks_sb = consts.tile([128, 512], BF16)
        wcol_sb = consts.tile([HD, HPC], F32)
        bsel_sb = consts.tile([128, 2], F32)
        ones_col = consts.tile([128, 1], F32)
        nc.vector.memset(ones_col, 1.0)
        ones_colb = consts.tile([128, 1], BF16)
        nc.vector.memset(ones_colb, 1.0)
        ones_row = consts.tile([1, 128], F32)
        nc.vector.memset(ones_row, 1.0)
        eps_sb = consts.tile([1, 1], F32)
        nc.vector.memset(eps_sb, EPS)
        warm_sb = consts.tile([1, 8], F32)
        nc.vector.memset(warm_sb, 0.0)

        a2a_in = [dram.tile([2 * TP, HD, 512], BF16, name=f"a2a_in{h}",
                            tag=f"a2a_in{h}") for h in range(HPC)]
        a2a_out = [dram.tile([2 * TP, HD, 512], BF16, name=f"a2a_out{h}",
                             tag=f"a2a_out{h}") for h in range(HPC)]
        ssq_in = [dram.tile([1, 2, 512], F32, name=f"ssq_in{t}",
                            tag=f"ssq_in{t}") for t in range(4)]
        ssq_out = [dram.tile([1, 2, 512], F32, name=f"ssq_out{t}",
                             tag=f"ssq_out{t}") for t in range(4)]
        warm_in = dram.tile([1, 8], F32, name="warm_in", tag="warm_in")
        warm_out = dram.tile([1, 8], F32, name="warm_out", tag="warm_out")

        persist = outer.enter_context(tc.tile_pool(name="persist", bufs=1))
        qt_f = persist.tile([128, HPC, S], BF16)   # normed q
        kt_f = persist.tile([128, S], BF16)        # normed k
        v_sb = persist.tile([128, NT, HD], BF16)
        gt_sb = persist.tile([128, NH, 512], BF16)
        # attention exp tiles, alternated by head parity (sliced to 4m+4 blocks)
        et_bufs = [persist.tile([128, NT, 512], BF16, name=f"etb{i}")
                   for i in range(2)]

        # one PSUM pool spanning the whole interleaved program
        mm_cm = tc.tile_pool(name="mmps", bufs=1, space="PSUM")
        mmps = mm_cm.__enter__()

        def big_ps():
            return mmps.tile([128, 512], F32, name="big", tag="big", bufs=3)

        def v_ps(shape=None):
            return mmps.tile(shape or [128, 512], F32, name="vps", tag="vps",
                             bufs=2)

        def row_ps():
            return mmps.tile([1, 512], F32, name="rowp", tag="rowp", bufs=2)

        def bc_ps():
            return mmps.tile([128, 512], F32, name="bcp", tag="bcp", bufs=1)

        # pools that live through P1+P3
        p13 = ExitStack()
        rows = p13.enter_context(tc.tile_pool(name="rows", bufs=1))
        p1t = p13.enter_context(tc.tile_pool(name="p1t", bufs=2))
        proj = p13.enter_context(tc.tile_pool(name="proj", bufs=1))
        xtp = p13.enter_context(tc.tile_pool(name="xtp", bufs=1))
        eraw_p = p13.enter_context(tc.tile_pool(name="eraw", bufs=2))
        smal = p13.enter_context(tc.tile_pool(name="smal", bufs=3))
        atp = p13.enter_context(tc.tile_pool(name="atp", bufs=1))
        p5t = p13.enter_context(tc.tile_pool(name="p5t", bufs=1))

        # ---- startup DMAs: warm the collective channel, then weights + x0
        nc.sync.dma_start(out=warm_in, in_=warm_sb)
        nc.gpsimd.collective_compute(
            "AllReduce", AluOp.add, replica_groups=groups4,
            ins=[warm_in.opt()], outs=[warm_out.opt()])

        wq_sb = proj.tile([128, ND, HPC * HD], BF16)
        wk_sb = proj.tile([128, ND, HD], BF16)
        wv_sb = proj.tile([128, ND, HD], BF16)
        # x chunks staged as two 8-tile halves, double-buffered
        xh0 = [xtp.tile([128, 8, 512], BF16, name=f"xts{i}", tag="xts",
                        bufs=2) for i in range(2)]
        for q4 in range(4):
            for dt in range(4 * q4, 4 * q4 + 4):
                rsl = slice(128 * dt, 128 * (dt + 1))
                nc.sync.dma_start(out=wq_sb[:, dt, :], in_=wqt_d[rsl, :])
            nc.sync.dma_start(
                out=xh0[q4 // 2][:, 4 * (q4 % 2):4 * (q4 % 2) + 4, :],
                in_=xt_d[512 * q4:512 * (q4 + 1), 0:512].rearrange(
                    "(n p) t -> p n t", p=128))
        nc.sync.dma_start(
            out=wk_sb, in_=wkt_d.rearrange("(n p) e -> p n e", p=128))
        nc.sync.dma_start(
            out=wv_sb, in_=wvt_d.rearrange("(n p) e -> p n e", p=128))
        nc.sync.dma_start(out=f12_sb, in_=f12_d[:, :, :])
        nc.sync.dma_start(out=masks_sb, in_=masks_d[:, :])
        nc.sync.dma_start(out=wcol_sb, in_=wcol_d[:, :])
        nc.sync.dma_start(out=bsel_sb, in_=bsel_d[:, :])

        def load_xts(cols):
            halves = []
            for i in range(2):
                xh = xtp.tile([128, 8, 512], BF16, name=f"xts{i}", tag="xts",
                              bufs=2)
                nc.sync.dma_start(
                    out=xh,
                    in_=xt_d[1024 * i:1024 * (i + 1), cols].rearrange(
                        "(n p) t -> p n t", p=128))
                halves.append(xh)
            return [halves[dt // 8][:, dt % 8, :] for dt in range(ND)]

        def rope_emit(ps, dst, gcols):
            # dst = ev*F1 + swap64(ev)*F2, F tables indexed by position
            ev = p1t.tile([128, 512], F32, tag="ev")
            nc.scalar.copy(out=ev, in_=ps)
            evs = p1t.tile([128, 512], F32, tag="evs")
            nc.sync.dma_start(out=evs[0:64, :], in_=ev[64:128, :])
            nc.sync.dma_start(out=evs[64:128, :], in_=ev[0:64, :])
            m1 = p1t.tile([128, 512], F32, tag="m1")
            m2 = p1t.tile([128, 512], F32, tag="m2")
            nc.vector.tensor_mul(m1, ev, f12_sb[:, 0, gcols])
            nc.vector.tensor_mul(m2, evs, f12_sb[:, 1, gcols])
            nc.vector.tensor_tensor(out=dst, in0=m1, in1=m2, op=AluOp.add)

        def v_proj(t4, xtt):
            for tt in range(4):
                psv = v_ps([128, HD])
                for dt in range(ND):
                    nc.tensor.matmul(
                        psv, xtt[dt][:, 128 * tt:128 * (tt + 1)],
                        wv_sb[:, dt, :],
                        start=(dt == 0), stop=(dt == ND - 1))
                nc.scalar.copy(out=v_sb[:, 4 * t4 + tt, :], in_=psv)

        def ssq_ar(t4, qtn, ktn):
            sps = row_ps()
            for h in range(HPC):
                sq = p1t.tile([128, 512], BF16, tag="sq")
                nc.gpsimd.tensor_mul(sq, qtn[:, h, :], qtn[:, h, :])
                nc.tensor.matmul(sps, ones_colb, sq,
                                 start=(h == 0), stop=(h == HPC - 1))
            sq2 = rows.tile([1, 2, 512], F32, tag="sq2", name="sq2", bufs=2)
            nc.vector.tensor_copy(sq2[:, 0, :], sps)
            sps_k = row_ps()
            sqk = p1t.tile([128, 512], BF16, tag="sq")
            nc.gpsimd.tensor_mul(sqk, ktn, ktn)
            nc.tensor.matmul(sps_k, ones_colb, sqk, start=True, stop=True)
            nc.vector.tensor_copy(sq2[:, 1, :], sps_k)
            nc.sync.dma_start(out=ssq_in[t4][:, :, :], in_=sq2)
            nc.gpsimd.collective_compute(
                "AllReduce", AluOp.add, replica_groups=groups4,
                ins=[ssq_in[t4].opt()], outs=[ssq_out[t4].opt()])

        def proj_chunk(t4, xtt):
            cols = slice(512 * t4, 512 * (t4 + 1))
            qtn = p1t.tile([128, HPC, 512], BF16, tag="qtn", bufs=2)
            ktn = p1t.tile([128, 512], BF16, tag="ktn", bufs=2)
            for h in range(HPC):
                ps = big_ps()
                for dt in range(ND):
                    nc.tensor.matmul(
                        ps, wq_sb[:, dt, HD * h:HD * (h + 1)], xtt[dt],
                        start=(dt == 0), stop=(dt == ND - 1))
                rope_emit(ps, qtn[:, h, :], cols)
            ps = big_ps()
            for dt in range(ND):
                nc.tensor.matmul(ps, wk_sb[:, dt, :], xtt[dt],
                                 start=(dt == 0), stop=(dt == ND - 1))
            rope_emit(ps, ktn, cols)
            ssq_ar(t4, qtn, ktn)
            v_proj(t4, xtt)
            return qtn, ktn

        def chunk_norm(t4, qtn, ktn):
            # rr = exp(-0.5*ln(ssq/n + eps)); q gets gamma column too
            cols = slice(512 * t4, 512 * (t4 + 1))
            rs_sb = rows.tile([1, 2, 512], F32, tag="rs", name="rs", bufs=2)
            nc.sync.dma_start(out=rs_sb, in_=ssq_out[t4][:, :, :])
            if debug:
                nc.sync.dma_start(out=dbg["ssq"][:, cols],
                                  in_=rs_sb.rearrange("p r s -> (p r) s"))
            tmp2 = rows.tile([1, 2, 512], F32, tag="lg", name="lg", bufs=2)
            rr2 = rows.tile([1, 2, 512], F32, tag="rr", name="rr", bufs=2)
            nc.scalar.activation(out=tmp2[:, 0, :], in_=rs_sb[:, 0, :],
                                 func=AFT.Ln, scale=1.0 / (NH * HD),
                                 bias=eps_sb)
            nc.scalar.activation(out=tmp2[:, 1, :], in_=rs_sb[:, 1, :],
                                 func=AFT.Ln, scale=1.0 / (NKV * HD),
                                 bias=eps_sb)
            nc.scalar.activation(out=rr2[:, 0, :], in_=tmp2[:, 0, :],
                                 func=AFT.Exp, scale=-0.5)
            nc.scalar.activation(out=rr2[:, 1, :], in_=tmp2[:, 1, :],
                                 func=AFT.Exp, scale=-0.5)
            bq = bc_ps()
            nc.tensor.matmul(bq, ones_row, rr2[:, 0, :], start=True, stop=True)
            rq_bc = smal.tile([128, 512], F32, tag="bcast")
            nc.vector.tensor_copy(rq_bc, bq)
            bk = bc_ps()
            nc.tensor.matmul(bk, ones_row, rr2[:, 1, :], start=True, stop=True)
            rk_bc = smal.tile([128, 512], F32, tag="bcast")
            nc.vector.tensor_copy(rk_bc, bk)
            for h in range(HPC):
                nc.vector.scalar_tensor_tensor(
                    out=qt_f[:, h, cols], in0=qtn[:, h, :],
                    scalar=wcol_sb[:, h:h + 1], in1=rq_bc,
                    op0=AluOp.mult, op1=AluOp.mult)
            nc.vector.tensor_tensor(out=kt_f[:, cols], in0=ktn,
                                    in1=rk_bc, op=AluOp.mult)

        # ---------------- attention round (heads pipelined, reduce
        # matmuls interleaved into the next head's score stream) ----------
        def attn_reduce_step(pend, kb, nkb):
            h, et, dn, at_ps = pend
            m = nkb // 4 - 1
            o = kb - 4 * m
            w = 512 if o < 0 else 512 - 128 * o
            nc.tensor.matmul(dn[:, 512 - w:], ones_colb,
                             et[:, kb, 512 - w:],
                             start=(kb == 0), stop=(kb == nkb - 1))
            nc.tensor.matmul(at_ps[:, 512 - w:], v_sb[:, kb, :],
                             et[:, kb, 512 - w:],
                             start=(kb == 0), stop=(kb == nkb - 1))

        def attn_epilogue(pend, m):
            h, et, dn, at_ps = pend
            rln = rows.tile([1, 512], F32, tag="rln", name="rln", bufs=2)
            nc.scalar.activation(out=rln, in_=dn, func=AFT.Ln, scale=1.0)
            rd = rows.tile([1, 512], F32, tag="rd", name="rd", bufs=2)
            nc.scalar.activation(out=rd, in_=rln, func=AFT.Exp, scale=-1.0)
            bc = bc_ps()
            nc.tensor.matmul(bc, ones_row, rd, start=True, stop=True)
            rdb = smal.tile([128, 512], F32, tag="bcast")
            nc.vector.tensor_copy(rdb, bc)
            ats = atp.tile([128, 512], BF16, tag="ats")
            nc.vector.tensor_mul(ats, at_ps, rdb)
            at0 = atp.tile([128, 512], BF16, tag="at0")
            at1 = atp.tile([128, 512], BF16, tag="at1")
            nc.gpsimd.tensor_scalar_mul(at0, ats, bsel_sb[:, 0:1])
            nc.gpsimd.tensor_scalar_mul(at1, ats, bsel_sb[:, 1:2])
            nc.sync.dma_start(out=a2a_in[h][m, :, :], in_=at0)
            nc.sync.dma_start(out=a2a_in[h][TP + m, :, :], in_=at1)
            if m == 3:
                nc.gpsimd.collective_compute(
                    "AllToAll", AluOp.bypass, replica_groups=groups8,
                    ins=[a2a_in[h].opt()], outs=[a2a_out[h].opt()])
                for i in range(TP):
                    sA = p5t.tile([128, 512], BF16, tag="sA")
                    sB = p5t.tile([128, 512], BF16, tag="sB")
                    nc.sync.dma_start(out=sA, in_=a2a_out[h][i, :, :])
                    nc.sync.dma_start(out=sB, in_=a2a_out[h][TP + i, :, :])
                    nc.gpsimd.tensor_tensor(out=gt_sb[:, 4 * i + h, :],
                                            in0=sA, in1=sB, op=AluOp.add)

        def attn_round(m):
            nkb = 4 * m + 4
            qc0 = 512 * m
            pend = None
            for h in range(HPC):
                et = et_bufs[h % 2]
                for kb in range(nkb):
                    o = kb - 4 * m
                    w = 512 if o < 0 else 512 - 128 * o
                    st = big_ps()
                    nc.tensor.matmul(
                        st[:, 512 - w:], kt_f[:, 128 * kb:128 * (kb + 1)],
                        qt_f[:, h, qc0 + 512 - w:qc0 + 512],
                        start=True, stop=True)
                    if pend is not None:
                        attn_reduce_step(pend, kb, nkb)
                    if o >= 0:
                        er = eraw_p.tile([128, 512], BF16, tag="eraw")
                        nc.scalar.activation(out=er[:, 0:w],
                                             in_=st[:, 512 - w:512],
                                             func=AFT.Exp, scale=SCALE)
                        nc.vector.tensor_mul(
                            et[:, kb, 512 - w:512], er[:, 0:w],
                            masks_sb[:, 0:w])
                    else:
                        nc.scalar.activation(out=et[:, kb, :], in_=st,
                                             func=AFT.Exp, scale=SCALE)
                if pend is not None:
                    attn_epilogue(pend, m)
                pend = (h, et, row_ps(), v_ps())
            # drain the last head (no next score stream to interleave into)
            for kb in range(nkb):
                attn_reduce_step(pend, kb, nkb)
            attn_epilogue(pend, m)

        # ------------------------ interleaved schedule --------------------
        xtt0 = [xh0[dt // 8][:, dt % 8, :] for dt in range(ND)]
        qk0 = proj_chunk(0, xtt0)
        qk1 = proj_chunk(1, load_xts(slice(512, 1024)))
        chunk_norm(0, *qk0)
        qk2 = proj_chunk(2, load_xts(slice(1024, 1536)))
        chunk_norm(1, *qk1)
        attn_round(0)
        qk3 = proj_chunk(3, load_xts(slice(1536, 2048)))
        chunk_norm(2, *qk2)
        attn_round(1)
        attn_round(2)
        chunk_norm(3, *qk3)
        attn_round(3)

        if debug:
            nc.sync.dma_start(out=dbg["qt"][:, :, :], in_=qt_f)
            nc.sync.dma_start(out=dbg["kt"][:, :], in_=kt_f)
            nc.sync.dma_start(out=dbg["v"][:, :, :], in_=v_sb)
            nc.sync.dma_start(out=dbg["at"][:, :, :], in_=gt_sb)

        p13.close()
        mm_cm.__exit__(None, None, None)

        # ---------------- P5: output projection (wo streamed by dc) -------
        with ExitStack() as p5s:
            p5o = p5s.enter_context(tc.tile_pool(name="p5o", bufs=3))
            wop5 = p5s.enter_context(tc.tile_pool(name="wop5", bufs=3))
            pop = p5s.enter_context(
                tc.tile_pool(name="pop", bufs=2, space="PSUM"))
            for dc in range(4):
                wo_dc = wop5.tile([128, ND, 512], BF16, name="wo_dc",
                                  tag="wo_dc")
                nc.sync.dma_start(
                    out=wo_dc,
                    in_=wot_d[:, 512 * dc:512 * (dc + 1)].rearrange(
                        "(n p) e -> p n e", p=128))
                opsd = [pop.tile([128, 512], F32, name=f"ops{t}",
                                 tag=f"tt{t}", bufs=2) for t in range(4)]
                for idx, e16 in enumerate(
                        [4 * i + h for h in range(HPC) for i in range(TP)]):
                    for tt in range(4):
                        nc.tensor.matmul(
                            opsd[tt], gt_sb[:, e16, 128 * tt:128 * (tt + 1)],
                            wo_dc[:, e16, :],
                            start=(idx == 0), stop=(idx == NH - 1))
                for tt in range(4):
                    osb = p5o.tile([128, 512], F32, tag="osb")
                    nc.vector.tensor_copy(osb, opsd[tt])
                    nc.sync.dma_start(
                        out=out_d[128 * tt:128 * (tt + 1),
                                  512 * dc:512 * (dc + 1)],
                        in_=osb)

    _split_sync_waits(nc)
    return nc


# ------------------------------------------------------------- host side --
_PERM = np.concatenate([np.arange(0, HD, 2), np.arange(1, HD, 2)])


def _prep(inputs):
    x = np.asarray(inputs["x"], np.float32)
    fc = np.asarray(inputs["freq_cis"], np.float32)
    wq = np.asarray(inputs["wq"], np.float32)
    wk = np.asarray(inputs["wk"], np.float32)
    wv = np.asarray(inputs["wv"], np.float32)
    wo = np.asarray(inputs["wo"], np.float32)
    qnw = np.asarray(inputs["q_norm_w"], np.float32)
    knw = np.asarray(inputs["k_norm_w"], np.float32)

    wq_p = wq.reshape(NH, HD, DIM)[:, _PERM, :]
    wk_p = wk.reshape(NKV, HD, DIM)[:, _PERM, :]
    qnw_p = qnw.reshape(NH, HD)[:, _PERM]
    knw_p = knw.reshape(NKV, HD)[:, _PERM]

    xt = [np.ascontiguousarray(x[b].T).astype(BF) for b in range(B)]
    wqt = [np.ascontiguousarray(
        wq_p[4 * g:4 * (g + 1)].reshape(4 * HD, DIM).T).astype(BF)
        for g in range(TP)]
    wkt = [np.ascontiguousarray(wk_p[g].T).astype(BF) for g in range(TP)]
    wvt = [np.ascontiguousarray(wv[g * HD:(g + 1) * HD].T).astype(BF)
           for g in range(TP)]
    wot = np.ascontiguousarray(wo.T).astype(BF)

    # rope tables: F1 = [cos|cos], F2 = [-sin|sin] along the partition halves
    cos = fc[:, :, 0, 0].T          # [64, S]
    sin = fc[:, :, 1, 0].T          # [64, S]
    f1 = np.concatenate([cos, cos], axis=0)
    f2 = np.concatenate([-sin, sin], axis=0)
    f12 = np.ascontiguousarray(
        np.stack([f1, f2], axis=1)).astype(BF)            # [128, 2, S]

    wcol = []
    for g in range(TP):
        cols = np.empty((HD, HPC), np.float32)
        for hl in range(HPC):
            cols[:, hl] = qnw_p[4 * g + hl] * knw_p[g]
        wcol.append(np.ascontiguousarray(cols))

    k_idx = np.arange(128)[:, None]
    q_idx = np.arange(512)[None, :]
    masks = np.ascontiguousarray((k_idx <= q_idx).astype(BF))   # [128, 512]

    in_maps = []
    for c in range(8):
        b, g = divmod(c, TP)
        bsel = np.zeros((128, 2), np.float32)
        bsel[:, b] = 1.0
        in_maps.append({
            "xt": xt[b], "wqt": wqt[g], "wkt": wkt[g], "wvt": wvt[g],
            "wot": wot, "f12": f12, "wcol": wcol[g], "masks": masks,
            "bsel": bsel,
        })
    return in_maps


_GRAPH_CACHE = {}


def _get_graph(debug=False):
    key = bool(debug)
    if key not in _GRAPH_CACHE:
        _GRAPH_CACHE[key] = build_graph(debug=key)
    return _GRAPH_CACHE[key]


LAST_RESULT = None


def kernel(debug=False, _run_kwargs=None, **inputs):
    global LAST_RESULT
    from concourse.bass_utils import run_bass_kernel_spmd

    nc = _get_graph(debug=debug)
    in_maps = _prep(inputs)
    res = run_bass_kernel_spmd(nc, in_maps, core_ids=list(range(8)),
                               **(_run_kwargs or {}))
    LAST_RESULT = res
    out = np.empty((B, S, DIM), np.float32)
    for c in range(8):
        b, g = divmod(c, TP)
        out[b, TOK * g:TOK * (g + 1), :] = res.results[c]["out"]
    if debug:
        return out, res
    return out


# revision 31
# speedup vs baseline: 1.1288x; 1.1288x over previous
"""Trainium2 Bass kernel for nn_Attention_23364622090354.

Attention with RoPE + flat QK-RMSNorm + GQA (16 q heads, 4 kv heads) +
causal softmax. B=2, S=2048, DIM=2048, HD=128.

Sharding (8 NeuronCores = 2 batches x 4-way head tensor-parallel):
  core c -> batch b = c//4, head group g = c%4 (q heads 4g..4g+3, kv head g).
Every core runs the identical causal program. Collectives: per-chunk
AllReduce of sum-of-squares rows (RMSNorm over flattened heads), and a
per-head 8-core AllToAll of the attention output so each core runs the
output projection for its own 512 sequence rows.

This version interleaves the projection chunks with the attention rounds
(m-outer) so the PE never drains at a phase boundary, restricts causal
diagonal blocks to their valid columns, computes every reciprocal as
exp(-ln x) on the scalar engine (single activation table), does rope with
3 full-width vector ops + a DMA partition-swap, and spreads epilogue
element-wise work across vector/scalar/gpsimd.
"""
import copy

import numpy as np
import ml_dtypes

import concourse.bass as bass
import concourse.mybir as mybir
from concourse.tile import TileContext
from concourse.vector_clock import ScopedClock
from concourse import tile as _tile_mod

BF = ml_dtypes.bfloat16
F32, BF16 = mybir.dt.float32, mybir.dt.bfloat16

B, S, DIM = 2, 2048, 2048
NH, NKV, HD = 16, 4, 128
TP = 4
HPC = NH // TP            # q heads per core = 4
EPS = 1e-6
SCALE = float(HD) ** (-0.5)
NT = S // 128             # 16 token tiles
ND = DIM // 128           # 16 contraction tiles
TOK = S // TP             # 512 tokens owned per core after A2A

AluOp = mybir.AluOpType
AFT = mybir.ActivationFunctionType


# ---------------------------------------------------------------- patches --
_ws_counter = [0]


def _split_sync_waits(nc, limit=1):
    """This neuronxcc rejects >1 sem wait per instruction; move extras onto
    same-engine NoOps placed immediately before (engines run in order)."""
    tmpl = nc.sync.nop(nofuse=True, hint="waitsplit-template").ins
    for fn in nc.m.functions:
        for bb in fn.blocks:
            if tmpl in bb.instructions:
                bb.instructions.remove(tmpl)
    for fn in nc.m.functions:
        for bb in fn.blocks:
            out = []
            changed = False
            for inst in bb.instructions:
                si = inst.sync_info
                waits = list(si.on_wait) if si is not None and si.on_wait else []
                if len(waits) > limit:
                    for w in waits[:-limit]:
                        _ws_counter[0] += 1
                        nop = copy.copy(tmpl)
                        nop.name = f"I-waitsplit-{_ws_counter[0]}"
                        nop.engine = inst.engine
                        nop.sync_info = mybir.SyncInfo(on_wait=[w], on_update=[])
                        out.append(nop)
                    si.on_wait = waits[-limit:]
                    changed = True
                out.append(inst)
            if changed:
                try:
                    bb.instructions[:] = out
                except TypeError:
                    bb.instructions = out


def _patched_drain_and_barrier(self, tick_clock, wait_clock):
    """Kernel-tail drain with waits redistributed to 1-wait NOPs."""
    nc = self.nc
    probe = nc.sync.nop(nofuse=True, hint="drain_waits")
    wait_clock.add_sem_waits(probe.ins, ScopedClock({None: tick_clock.global_clock}))
    si = probe.ins.sync_info
    waits = list(si.on_wait or []) if si is not None else []
    if len(waits) > 1:
        si.on_wait = waits[:1]
        for w in waits[1:]:
            extra = nc.sync.nop(nofuse=True, hint="drain_waits")
            extra.ins.sync_info = mybir.SyncInfo(on_wait=[w], on_update=[])
    nc.sync.drain()
    nc.all_engine_barrier()
    assert self.sems is not None
    popped = nc._tile_sem_poison_stack.pop()
    assert popped is self._sem_poison
    nc.clear_and_free_semaphores(list(self.sems.allocated().values()))
    nc.all_engine_barrier()


_tile_mod.TileContext._drain_and_barrier = _patched_drain_and_barrier


# ------------------------------------------------------------------ graph --
def build_graph(debug=False):
    nc = bass.Bass()
    xt_d = nc.dram_tensor("xt", [DIM, S], BF16, kind="ExternalInput")
    wqt_d = nc.dram_tensor("wqt", [DIM, HPC * HD], BF16, kind="ExternalInput")
    wkt_d = nc.dram_tensor("wkt", [DIM, HD], BF16, kind="ExternalInput")
    wvt_d = nc.dram_tensor("wvt", [DIM, HD], BF16, kind="ExternalInput")
    wot_d = nc.dram_tensor("wot", [NH * HD, DIM], BF16, kind="ExternalInput")
    f12_d = nc.dram_tensor("f12", [128, 2, S], BF16, kind="ExternalInput")
    wcol_d = nc.dram_tensor("wcol", [HD, HPC], F32, kind="ExternalInput")
    masks_d = nc.dram_tensor("masks", [128, 512], BF16, kind="ExternalInput")
    bsel_d = nc.dram_tensor("bsel", [128, 2], F32, kind="ExternalInput")
    out_d = nc.dram_tensor("out", [TOK, DIM], F32, kind="ExternalOutput")
    dbg = {}
    if debug:
        dbg["qt"] = nc.dram_tensor("dbg_qt", [128, HPC, S], BF16, kind="ExternalOutput")
        dbg["kt"] = nc.dram_tensor("dbg_kt", [128, S], BF16, kind="ExternalOutput")
        dbg["v"] = nc.dram_tensor("dbg_v", [128, NT, HD], BF16, kind="ExternalOutput")
        dbg["ssq"] = nc.dram_tensor("dbg_ssq", [2, S], F32, kind="ExternalOutput")
        dbg["at"] = nc.dram_tensor("dbg_at", [128, NH, 512], BF16,
                                   kind="ExternalOutput")

    groups4 = [[0, 1, 2, 3], [4, 5, 6, 7]]
    groups8 = [list(range(8))]

    from contextlib import ExitStack
    with TileContext(nc) as tc, ExitStack() as outer:
        consts = outer.enter_context(tc.tile_pool(name="consts", bufs=1))
        dram = outer.enter_context(tc.tile_pool(name="dram", bufs=1, space="DRAM"))

        f12_sb = consts.tile([128, 2, S], BF16)
        masks_sb = consts.tile([128, 512], BF16)
        wcol_sb = consts.tile([HD, HPC], F32)
        bsel_sb = consts.tile([128, 2], F32)
        ones_col = consts.tile([128, 1], F32)
        nc.vector.memset(ones_col, 1.0)
        ones_colb = consts.tile([128, 1], BF16)
        nc.vector.memset(ones_colb, 1.0)
        ones_row = consts.tile([1, 128], F32)
        nc.vector.memset(ones_row, 1.0)
        eps_sb = consts.tile([1, 1], F32)
        nc.vector.memset(eps_sb, EPS)
        warm_sb = consts.tile([1, 8], F32)
        nc.vector.memset(warm_sb, 0.0)

        a2a_in = [dram.tile([2 * TP, HD, 512], BF16, name=f"a2a_in{h}",
                            tag=f"a2a_in{h}") for h in range(HPC)]
        a2a_out = [dram.tile([2 * TP, HD, 512], BF16, name=f"a2a_out{h}",
                             tag=f"a2a_out{h}") for h in range(HPC)]
        ssq_in = [dram.tile([1, 2, 512], F32, name=f"ssq_in{t}",
                            tag=f"ssq_in{t}") for t in range(4)]
        ssq_out = [dram.tile([1, 2, 512], F32, name=f"ssq_out{t}",
                             tag=f"ssq_out{t}") for t in range(4)]
        warm_in = dram.tile([1, 8], F32, name="warm_in", tag="warm_in")
        warm_out = dram.tile([1, 8], F32, name="warm_out", tag="warm_out")

        persist = outer.enter_context(tc.tile_pool(name="persist", bufs=1))
        qt_f = persist.tile([128, HPC, S], BF16)   # normed q
        kt_f = persist.tile([128, S], BF16)        # normed k
        v_sb = persist.tile([128, NT, HD], BF16)
        gt_sb = persist.tile([128, NH, 512], BF16)
        # attention exp tiles, alternated by head parity (sliced to 4m+4 blocks)
        et_bufs = [persist.tile([128, NT, 512], BF16, name=f"etb{i}")
                   for i in range(2)]

        # one PSUM pool spanning the whole interleaved program
        mm_cm = tc.tile_pool(name="mmps", bufs=1, space="PSUM")
        mmps = mm_cm.__enter__()

        def big_ps():
            return mmps.tile([128, 512], F32, name="big", tag="big", bufs=3)

        def v_ps(shape=None):
            return mmps.tile(shape or [128, 512], F32, name="vps", tag="vps",
                             bufs=2)

        def row_ps():
            return mmps.tile([1, 512], F32, name="rowp", tag="rowp", bufs=2)

        def bc_ps():
            return mmps.tile([128, 512], F32, name="bcp", tag="bcp", bufs=1)

        # pools that live through P1+P3
        p13 = ExitStack()
        rows = p13.enter_context(tc.tile_pool(name="rows", bufs=1))
        p1t = p13.enter_context(tc.tile_pool(name="p1t", bufs=2))
        proj = p13.enter_context(tc.tile_pool(name="proj", bufs=1))
        xtp = p13.enter_context(tc.tile_pool(name="xtp", bufs=1))
        eraw_p = p13.enter_context(tc.tile_pool(name="eraw", bufs=2))
        smal = p13.enter_context(tc.tile_pool(name="smal", bufs=3))
        atp = p13.enter_context(tc.tile_pool(name="atp", bufs=1))
        p5t = p13.enter_context(tc.tile_pool(name="p5t", bufs=1))

        # ---- startup DMAs: warm the collective channel, then weights + x0
        nc.sync.dma_start(out=warm_in, in_=warm_sb)
        nc.gpsimd.collective_compute(
            "AllReduce", AluOp.add, replica_groups=groups4,
            ins=[warm_in.opt()], outs=[warm_out.opt()])

        wq_sb = proj.tile([128, ND, HPC * HD], BF16)
        wk_sb = proj.tile([128, ND, HD], BF16)
        wv_sb = proj.tile([128, ND, HD], BF16)
        # x chunks staged as two 8-tile halves, double-buffered
        xh0 = [xtp.tile([128, 8, 512], BF16, name=f"xts{i}", tag="xts",
                        bufs=2) for i in range(2)]
        for q4 in range(4):
            for dt in range(4 * q4, 4 * q4 + 4):
                rsl = slice(128 * dt, 128 * (dt + 1))
                nc.sync.dma_start(out=wq_sb[:, dt, :], in_=wqt_d[rsl, :])
            nc.sync.dma_start(
                out=xh0[q4 // 2][:, 4 * (q4 % 2):4 * (q4 % 2) + 4, :],
                in_=xt_d[512 * q4:512 * (q4 + 1), 0:512].rearrange(
                    "(n p) t -> p n t", p=128))
        nc.sync.dma_start(
            out=wk_sb, in_=wkt_d.rearrange("(n p) e -> p n e", p=128))
        nc.sync.dma_start(
            out=wv_sb, in_=wvt_d.rearrange("(n p) e -> p n e", p=128))
        nc.sync.dma_start(out=f12_sb, in_=f12_d[:, :, :])
        nc.sync.dma_start(out=masks_sb, in_=masks_d[:, :])
        nc.sync.dma_start(out=wcol_sb, in_=wcol_d[:, :])
        nc.sync.dma_start(out=bsel_sb, in_=bsel_d[:, :])

        def load_xts(cols):
            halves = []
            for i in range(2):
                xh = xtp.tile([128, 8, 512], BF16, name=f"xts{i}", tag="xts",
                              bufs=2)
                nc.sync.dma_start(
                    out=xh,
                    in_=xt_d[1024 * i:1024 * (i + 1), cols].rearrange(
                        "(n p) t -> p n t", p=128))
                halves.append(xh)
            return [halves[dt // 8][:, dt % 8, :] for dt in range(ND)]

        def rope_emit(ps, dst, gcols):
            # dst = ev*F1 + swap64(ev)*F2, F tables indexed by position
            ev = p1t.tile([128, 512], F32, tag="ev")
            nc.scalar.copy(out=ev, in_=ps)
            evs = p1t.tile([128, 512], F32, tag="evs")
            nc.sync.dma_start(out=evs[0:64, :], in_=ev[64:128, :])
            nc.sync.dma_start(out=evs[64:128, :], in_=ev[0:64, :])
            m1 = p1t.tile([128, 512], F32, tag="m1")
            m2 = p1t.tile([128, 512], F32, tag="m2")
            nc.vector.tensor_mul(m1, ev, f12_sb[:, 0, gcols])
            nc.vector.tensor_mul(m2, evs, f12_sb[:, 1, gcols])
            nc.vector.tensor_tensor(out=dst, in0=m1, in1=m2, op=AluOp.add)

        def v_proj(t4, xtt):
            for tt in range(4):
                psv = v_ps([128, HD])
                for dt in range(ND):
                    nc.tensor.matmul(
                        psv, xtt[dt][:, 128 * tt:128 * (tt + 1)],
                        wv_sb[:, dt, :],
                        start=(dt == 0), stop=(dt == ND - 1))
                nc.scalar.copy(out=v_sb[:, 4 * t4 + tt, :], in_=psv)

        def ssq_ar(t4, qtn, ktn):
            sps = row_ps()
            for h in range(HPC):
                sq = p1t.tile([128, 512], BF16, tag="sq")
                nc.gpsimd.tensor_mul(sq, qtn[:, h, :], qtn[:, h, :])
                nc.tensor.matmul(sps, ones_colb, sq,
                                 start=(h == 0), stop=(h == HPC - 1))
            sq2 = rows.tile([1, 2, 512], F32, tag="sq2", name="sq2", bufs=2)
            nc.vector.tensor_copy(sq2[:, 0, :], sps)
            sps_k = row_ps()
            sqk = p1t.tile([128, 512], BF16, tag="sq")
            nc.gpsimd.tensor_mul(sqk, ktn, ktn)
            nc.tensor.matmul(sps_k, ones_colb, sqk, start=True, stop=True)
            nc.vector.tensor_copy(sq2[:, 1, :], sps_k)
            nc.sync.dma_start(out=ssq_in[t4][:, :, :], in_=sq2)
            nc.gpsimd.collective_compute(
                "AllReduce", AluOp.add, replica_groups=groups4,
                ins=[ssq_in[t4].opt()], outs=[ssq_out[t4].opt()])

        def proj_chunk(t4, xtt):
            cols = slice(512 * t4, 512 * (t4 + 1))
            qtn = p1t.tile([128, HPC, 512], BF16, tag="qtn", bufs=2)
            ktn = p1t.tile([128, 512], BF16, tag="ktn", bufs=2)
            for h in range(HPC):
                ps = big_ps()
                for dt in range(ND):
                    nc.tensor.matmul(
                        ps, wq_sb[:, dt, HD * h:HD * (h + 1)], xtt[dt],
                        start=(dt == 0), stop=(dt == ND - 1))
                rope_emit(ps, qtn[:, h, :], cols)
            ps = big_ps()
            for dt in range(ND):
                nc.tensor.matmul(ps, wk_sb[:, dt, :], xtt[dt],
                                 start=(dt == 0), stop=(dt == ND - 1))
            rope_emit(ps, ktn, cols)
            ssq_ar(t4, qtn, ktn)
            v_proj(t4, xtt)
            return qtn, ktn

        def chunk_norm(t4, qtn, ktn):
            # rr = exp(-0.5*ln(ssq/n + eps)); q gets gamma column too
            cols = slice(512 * t4, 512 * (t4 + 1))
            rs_sb = rows.tile([1, 2, 512], F32, tag="rs", name="rs", bufs=2)
            nc.sync.dma_start(out=rs_sb, in_=ssq_out[t4][:, :, :])
            if debug:
                nc.sync.dma_start(out=dbg["ssq"][:, cols],
                                  in_=rs_sb.rearrange("p r s -> (p r) s"))
            tmp2 = rows.tile([1, 2, 512], F32, tag="lg", name="lg", bufs=2)
            rr2 = rows.tile([1, 2, 512], F32, tag="rr", name="rr", bufs=2)
            nc.scalar.activation(out=tmp2[:, 0, :], in_=rs_sb[:, 0, :],
                                 func=AFT.Ln, scale=1.0 / (NH * HD),
                                 bias=eps_sb)
            nc.scalar.activation(out=tmp2[:, 1, :], in_=rs_sb[:, 1, :],
                                 func=AFT.Ln, scale=1.0 / (NKV * HD),
                                 bias=eps_sb)
            nc.scalar.activation(out=rr2[:, 0, :], in_=tmp2[:, 0, :],
                                 func=AFT.Exp, scale=-0.5)
            nc.scalar.activation(out=rr2[:, 1, :], in_=tmp2[:, 1, :],
                                 func=AFT.Exp, scale=-0.5)
            bq = bc_ps()
            nc.tensor.matmul(bq, ones_row, rr2[:, 0, :], start=True, stop=True)
            rq_bc = smal.tile([128, 512], F32, tag="bcast")
            nc.vector.tensor_copy(rq_bc, bq)
            bk = bc_ps()
            nc.tensor.matmul(bk, ones_row, rr2[:, 1, :], start=True, stop=True)
            rk_bc = smal.tile([128, 512], F32, tag="bcast")
            nc.vector.tensor_copy(rk_bc, bk)
            for h in range(HPC):
                nc.vector.scalar_tensor_tensor(
                    out=qt_f[:, h, cols], in0=qtn[:, h, :],
                    scalar=wcol_sb[:, h:h + 1], in1=rq_bc,
                    op0=AluOp.mult, op1=AluOp.mult)
            nc.vector.tensor_tensor(out=kt_f[:, cols], in0=ktn,
                                    in1=rk_bc, op=AluOp.mult)

        # ---------------- attention round (heads pipelined, reduce
        # matmuls interleaved into the next head's score stream) ----------
        def attn_reduce_step(pend, kb, nkb):
            h, et, dn, at_ps = pend
            m = nkb // 4 - 1
            o = kb - 4 * m
            w = 512 if o < 0 else 512 - 128 * o
            nc.tensor.matmul(dn[:, 512 - w:], ones_colb,
                             et[:, kb, 512 - w:],
                             start=(kb == 0), stop=(kb == nkb - 1))
            nc.tensor.matmul(at_ps[:, 512 - w:], v_sb[:, kb, :],
                             et[:, kb, 512 - w:],
                             start=(kb == 0), stop=(kb == nkb - 1))

        def attn_epilogue(pend, m):
            h, et, dn, at_ps = pend
            rln = rows.tile([1, 512], F32, tag="rln", name="rln", bufs=2)
            nc.scalar.activation(out=rln, in_=dn, func=AFT.Ln, scale=1.0)
            rd = rows.tile([1, 512], F32, tag="rd", name="rd", bufs=2)
            nc.scalar.activation(out=rd, in_=rln, func=AFT.Exp, scale=-1.0)
            bc = bc_ps()
            nc.tensor.matmul(bc, ones_row, rd, start=True, stop=True)
            rdb = smal.tile([128, 512], F32, tag="bcast")
            nc.vector.tensor_copy(rdb, bc)
            at0 = atp.tile([128, 512], BF16, tag="at0")
            at1 = atp.tile([128, 512], BF16, tag="at1")
            nc.vector.scalar_tensor_tensor(
                out=at0, in0=at_ps, scalar=bsel_sb[:, 0:1], in1=rdb,
                op0=AluOp.mult, op1=AluOp.mult)
            nc.vector.scalar_tensor_tensor(
                out=at1, in0=at_ps, scalar=bsel_sb[:, 1:2], in1=rdb,
                op0=AluOp.mult, op1=AluOp.mult)
            nc.sync.dma_start(out=a2a_in[h][m, :, :], in_=at0)
            nc.sync.dma_start(out=a2a_in[h][TP + m, :, :], in_=at1)
            if m == 3:
                nc.gpsimd.collective_compute(
                    "AllToAll", AluOp.bypass, replica_groups=groups8,
                    ins=[a2a_in[h].opt()], outs=[a2a_out[h].opt()])
                for i in range(TP):
                    sA = p5t.tile([128, 512], BF16, tag="sA")
                    sB = p5t.tile([128, 512], BF16, tag="sB")
                    nc.sync.dma_start(out=sA, in_=a2a_out[h][i, :, :])
                    nc.sync.dma_start(out=sB, in_=a2a_out[h][TP + i, :, :])
                    nc.gpsimd.tensor_tensor(out=gt_sb[:, 4 * i + h, :],
                                            in0=sA, in1=sB, op=AluOp.add)

        def attn_round(m):
            nkb = 4 * m + 4
            qc0 = 512 * m
            pend = None
            for h in range(HPC):
                et = et_bufs[h % 2]
                for kb in range(nkb):
                    o = kb - 4 * m
                    w = 512 if o < 0 else 512 - 128 * o
                    st = big_ps()
                    nc.tensor.matmul(
                        st[:, 512 - w:], kt_f[:, 128 * kb:128 * (kb + 1)],
                        qt_f[:, h, qc0 + 512 - w:qc0 + 512],
                        start=True, stop=True)
                    if pend is not None:
                        attn_reduce_step(pend, kb, nkb)
                    if o >= 0:
                        er = eraw_p.tile([128, 512], BF16, tag="eraw")
                        nc.scalar.activation(out=er[:, 0:w],
                                             in_=st[:, 512 - w:512],
                                             func=AFT.Exp, scale=SCALE)
                        nc.vector.tensor_mul(
                            et[:, kb, 512 - w:512], er[:, 0:w],
                            masks_sb[:, 0:w])
                    else:
                        nc.scalar.activation(out=et[:, kb, :], in_=st,
                                             func=AFT.Exp, scale=SCALE)
                if pend is not None:
                    attn_epilogue(pend, m)
                pend = (h, et, row_ps(), v_ps())
            # drain the last head (no next score stream to interleave into)
            for kb in range(nkb):
                attn_reduce_step(pend, kb, nkb)
            attn_epilogue(pend, m)

        # ------------------------ interleaved schedule --------------------
        def wo_half_tiles(dc):
            xw = []
            for i in range(2):
                xh = xtp.tile([128, 8, 512], BF16, name=f"xts{i}", tag="xts",
                              bufs=2)
                nc.sync.dma_start(
                    out=xh,
                    in_=wot_d[1024 * i:1024 * (i + 1),
                              512 * dc:512 * (dc + 1)].rearrange(
                        "(n p) e -> p n e", p=128))
                xw.append(xh)
            return [xw[n // 8][:, n % 8, :] for n in range(NH)]

        xtt0 = [xh0[dt // 8][:, dt % 8, :] for dt in range(ND)]
        qk0 = proj_chunk(0, xtt0)
        qk1 = proj_chunk(1, load_xts(slice(512, 1024)))
        chunk_norm(0, *qk0)
        qk2 = proj_chunk(2, load_xts(slice(1024, 1536)))
        chunk_norm(1, *qk1)
        attn_round(0)
        qk3 = proj_chunk(3, load_xts(slice(1536, 2048)))
        # prefetch wo dc0 into wq_sb once the last projection released it
        nc.sync.dma_start(
            out=wq_sb,
            in_=wot_d[:, 0:512].rearrange("(n p) e -> p n e", p=128))
        chunk_norm(2, *qk2)
        attn_round(1)
        attn_round(2)
        chunk_norm(3, *qk3)
        wo1 = wo_half_tiles(1)     # prefetch wo dc1 during round 3
        attn_round(3)

        if debug:
            nc.sync.dma_start(out=dbg["qt"][:, :, :], in_=qt_f)
            nc.sync.dma_start(out=dbg["kt"][:, :], in_=kt_f)
            nc.sync.dma_start(out=dbg["v"][:, :, :], in_=v_sb)
            nc.sync.dma_start(out=dbg["at"][:, :, :], in_=gt_sb)

        mm_cm.__exit__(None, None, None)

        # ---------------- P5: output projection (wo pre-staged) -----------
        with tc.tile_pool(name="pop", bufs=2, space="PSUM") as pop:
            for dc in range(4):
                if dc == 0:
                    wo_view = [wq_sb[:, n, :] for n in range(NH)]
                elif dc == 1:
                    wo_view = wo1
                elif dc == 2:
                    nc.sync.dma_start(
                        out=wq_sb,
                        in_=wot_d[:, 1024:1536].rearrange(
                            "(n p) e -> p n e", p=128))
                    wo_view = [wq_sb[:, n, :] for n in range(NH)]
                else:
                    wo_view = wo_half_tiles(3)
                opsd = [pop.tile([128, 512], F32, name=f"ops{t}",
                                 tag=f"tt{t}", bufs=2) for t in range(4)]
                for idx, e16 in enumerate(
                        [4 * i + h for h in range(HPC) for i in range(TP)]):
                    for tt in range(4):
                        nc.tensor.matmul(
                            opsd[tt], gt_sb[:, e16, 128 * tt:128 * (tt + 1)],
                            wo_view[e16],
                            start=(idx == 0), stop=(idx == NH - 1))
                for tt in range(4):
                    osb = smal.tile([128, 512], F32, tag="bcast")
                    nc.vector.tensor_copy(osb, opsd[tt])
                    nc.sync.dma_start(
                        out=out_d[128 * tt:128 * (tt + 1),
                                  512 * dc:512 * (dc + 1)],
                        in_=osb)
        p13.close()

    _split_sync_waits(nc)
    return nc


# ------------------------------------------------------------- host side --
_PERM = np.concatenate([np.arange(0, HD, 2), np.arange(1, HD, 2)])


def _prep(inputs):
    x = np.asarray(inputs["x"], np.float32)
    fc = np.asarray(inputs["freq_cis"], np.float32)
    wq = np.asarray(inputs["wq"], np.float32)
    wk = np.asarray(inputs["wk"], np.float32)
    wv = np.asarray(inputs["wv"], np.float32)
    wo = np.asarray(inputs["wo"], np.float32)
    qnw = np.asarray(inputs["q_norm_w"], np.float32)
    knw = np.asarray(inputs["k_norm_w"], np.float32)

    wq_p = wq.reshape(NH, HD, DIM)[:, _PERM, :]
    wk_p = wk.reshape(NKV, HD, DIM)[:, _PERM, :]
    qnw_p = qnw.reshape(NH, HD)[:, _PERM]
    knw_p = knw.reshape(NKV, HD)[:, _PERM]

    xt = [np.ascontiguousarray(x[b].T).astype(BF) for b in range(B)]
    wqt = [np.ascontiguousarray(
        wq_p[4 * g:4 * (g + 1)].reshape(4 * HD, DIM).T).astype(BF)
        for g in range(TP)]
    wkt = [np.ascontiguousarray(wk_p[g].T).astype(BF) for g in range(TP)]
    wvt = [np.ascontiguousarray(wv[g * HD:(g + 1) * HD].T).astype(BF)
           for g in range(TP)]
    wot = np.ascontiguousarray(wo.T).astype(BF)

    # rope tables: F1 = [cos|cos], F2 = [-sin|sin] along the partition halves
    cos = fc[:, :, 0, 0].T          # [64, S]
    sin = fc[:, :, 1, 0].T          # [64, S]
    f1 = np.concatenate([cos, cos], axis=0)
    f2 = np.concatenate([-sin, sin], axis=0)
    f12 = np.ascontiguousarray(
        np.stack([f1, f2], axis=1)).astype(BF)            # [128, 2, S]

    wcol = []
    for g in range(TP):
        cols = np.empty((HD, HPC), np.float32)
        for hl in range(HPC):
            cols[:, hl] = qnw_p[4 * g + hl] * knw_p[g]
        wcol.append(np.ascontiguousarray(cols))

    k_idx = np.arange(128)[:, None]
    q_idx = np.arange(512)[None, :]
    masks = np.ascontiguousarray((k_idx <= q_idx).astype(BF))   # [128, 512]

    in_maps = []
    for c in range(8):
        b, g = divmod(c, TP)
        bsel = np.zeros((128, 2), np.float32)
        bsel[:, b] = 1.0
        in_maps.append({
            "xt": xt[b], "wqt": wqt[g], "wkt": wkt[g], "wvt": wvt[g],
            "wot": wot, "f12": f12, "wcol": wcol[g], "masks": masks,
            "bsel": bsel,
        })
    return in_maps


_GRAPH_CACHE = {}


def _get_graph(debug=False):
    key = bool(debug)
    if key not in _GRAPH_CACHE:
        _GRAPH_CACHE[key] = build_graph(debug=key)
    return _GRAPH_CACHE[key]


LAST_RESULT = None


def kernel(debug=False, _run_kwargs=None, **inputs):
    global LAST_RESULT
    from concourse.bass_utils import run_bass_kernel_spmd

    nc = _get_graph(debug=debug)
    in_maps = _prep(inputs)
    res = run_bass_kernel_spmd(nc, in_maps, core_ids=list(range(8)),
                               **(_run_kwargs or {}))
    LAST_RESULT = res
    out = np.empty((B, S, DIM), np.float32)
    for c in range(8):
        b, g = divmod(c, TP)
        out[b, TOK * g:TOK * (g + 1), :] = res.results[c]["out"]
    if debug:
        return out, res
    return out


# revision 41
# speedup vs baseline: 1.3131x; 1.1633x over previous
"""Trainium2 Bass kernel for nn_Attention_23364622090354.

Attention with RoPE + flat QK-RMSNorm + GQA (16 q heads, 4 kv heads) +
causal softmax. B=2, S=2048, DIM=2048, HD=128.

Sharding (8 NeuronCores = 2 batches x 4-way head tensor-parallel):
  core c -> batch b = c//4, head group g = c%4 (q heads 4g..4g+3, kv head g).
Every core runs the identical causal program. Collectives: per-chunk
AllReduce of sum-of-squares rows (RMSNorm over flattened heads), and a
per-head 8-core AllToAll of the attention output so each core runs the
output projection for its own 512 sequence rows.

This version interleaves the projection chunks with the attention rounds
(m-outer) so the PE never drains at a phase boundary, restricts causal
diagonal blocks to their valid columns, computes every reciprocal as
exp(-ln x) on the scalar engine (single activation table), does rope with
3 full-width vector ops + a DMA partition-swap, and spreads epilogue
element-wise work across vector/scalar/gpsimd.
"""
import copy

import numpy as np
import ml_dtypes

import concourse.bass as bass
import concourse.mybir as mybir
from concourse.tile import TileContext
from concourse.vector_clock import ScopedClock
from concourse import tile as _tile_mod

BF = ml_dtypes.bfloat16
F32, BF16 = mybir.dt.float32, mybir.dt.bfloat16

B, S, DIM = 2, 2048, 2048
NH, NKV, HD = 16, 4, 128
TP = 4
HPC = NH // TP            # q heads per core = 4
EPS = 1e-6
SCALE = float(HD) ** (-0.5)
NT = S // 128             # 16 token tiles
ND = DIM // 128           # 16 contraction tiles
TOK = S // TP             # 512 tokens owned per core after A2A

AluOp = mybir.AluOpType
AFT = mybir.ActivationFunctionType


# ---------------------------------------------------------------- patches --
_ws_counter = [0]


def _split_sync_waits(nc, limit=1):
    """This neuronxcc rejects >1 sem wait per instruction; move extras onto
    same-engine NoOps placed immediately before (engines run in order)."""
    tmpl = nc.sync.nop(nofuse=True, hint="waitsplit-template").ins
    for fn in nc.m.functions:
        for bb in fn.blocks:
            if tmpl in bb.instructions:
                bb.instructions.remove(tmpl)
    for fn in nc.m.functions:
        for bb in fn.blocks:
            out = []
            changed = False
            for inst in bb.instructions:
                si = inst.sync_info
                waits = list(si.on_wait) if si is not None and si.on_wait else []
                if len(waits) > limit:
                    for w in waits[:-limit]:
                        _ws_counter[0] += 1
                        nop = copy.copy(tmpl)
                        nop.name = f"I-waitsplit-{_ws_counter[0]}"
                        nop.engine = inst.engine
                        nop.sync_info = mybir.SyncInfo(on_wait=[w], on_update=[])
                        out.append(nop)
                    si.on_wait = waits[-limit:]
                    changed = True
                out.append(inst)
            if changed:
                try:
                    bb.instructions[:] = out
                except TypeError:
                    bb.instructions = out


def _patched_drain_and_barrier(self, tick_clock, wait_clock):
    """Kernel-tail drain with waits redistributed to 1-wait NOPs."""
    nc = self.nc
    probe = nc.sync.nop(nofuse=True, hint="drain_waits")
    wait_clock.add_sem_waits(probe.ins, ScopedClock({None: tick_clock.global_clock}))
    si = probe.ins.sync_info
    waits = list(si.on_wait or []) if si is not None else []
    if len(waits) > 1:
        si.on_wait = waits[:1]
        for w in waits[1:]:
            extra = nc.sync.nop(nofuse=True, hint="drain_waits")
            extra.ins.sync_info = mybir.SyncInfo(on_wait=[w], on_update=[])
    nc.sync.drain()
    nc.all_engine_barrier()
    assert self.sems is not None
    popped = nc._tile_sem_poison_stack.pop()
    assert popped is self._sem_poison
    nc.clear_and_free_semaphores(list(self.sems.allocated().values()))
    nc.all_engine_barrier()


_tile_mod.TileContext._drain_and_barrier = _patched_drain_and_barrier


# ------------------------------------------------------------------ graph --
def build_graph(debug=False):
    nc = bass.Bass()
    xt_d = nc.dram_tensor("xt", [DIM, S], BF16, kind="ExternalInput")
    wqt_d = nc.dram_tensor("wqt", [DIM, HPC * HD], BF16, kind="ExternalInput")
    wkt_d = nc.dram_tensor("wkt", [DIM, HD], BF16, kind="ExternalInput")
    wvt_d = nc.dram_tensor("wvt", [DIM, HD], BF16, kind="ExternalInput")
    wot_d = nc.dram_tensor("wot", [NH * HD, DIM], BF16, kind="ExternalInput")
    f12_d = nc.dram_tensor("f12", [128, 2, S], BF16, kind="ExternalInput")
    wcol_d = nc.dram_tensor("wcol", [HD, HPC], F32, kind="ExternalInput")
    utri_d = nc.dram_tensor("utri", [128, 512], BF16, kind="ExternalInput")
    ltri_d = nc.dram_tensor("ltri", [128, 128], BF16, kind="ExternalInput")
    bsel_d = nc.dram_tensor("bsel", [128, 2], F32, kind="ExternalInput")
    out_d = nc.dram_tensor("out", [TOK, DIM], F32, kind="ExternalOutput")
    dbg = {}
    if debug:
        dbg["qt"] = nc.dram_tensor("dbg_qt", [128, HPC, S], BF16, kind="ExternalOutput")
        dbg["kt"] = nc.dram_tensor("dbg_kt", [128, S], BF16, kind="ExternalOutput")
        dbg["v"] = nc.dram_tensor("dbg_v", [128, NT, HD], BF16, kind="ExternalOutput")
        dbg["ssq"] = nc.dram_tensor("dbg_ssq", [2, S], F32, kind="ExternalOutput")
        dbg["at"] = nc.dram_tensor("dbg_at", [128, NH, 512], BF16,
                                   kind="ExternalOutput")

    groups4 = [[0, 1, 2, 3], [4, 5, 6, 7]]
    groups8 = [list(range(8))]

    from contextlib import ExitStack
    with TileContext(nc) as tc, ExitStack() as outer:
        consts = outer.enter_context(tc.tile_pool(name="consts", bufs=1))
        dram = outer.enter_context(tc.tile_pool(name="dram", bufs=1, space="DRAM"))

        f12_sb = consts.tile([128, 2, S], BF16)
        # utri[j, q] = -1000 for j > q else 0; ltri = identity, so the extra
        # matmul accumulates a -1000 step above the block diagonal and the
        # exp underflows those scores to zero -- no separate mask multiply.
        utri_sb = consts.tile([128, 512], BF16)
        ltri_sb = consts.tile([128, 128], BF16)
        wcol_sb = consts.tile([HD, HPC], F32)
        bsel_sb = consts.tile([128, 2], F32)
        ones_col = consts.tile([128, 1], F32)
        nc.vector.memset(ones_col, 1.0)
        ones_colb = consts.tile([128, 1], BF16)
        nc.vector.memset(ones_colb, 1.0)
        ones_row = consts.tile([1, 128], F32)
        nc.vector.memset(ones_row, 1.0)
        eps_sb = consts.tile([1, 1], F32)
        nc.vector.memset(eps_sb, EPS)
        warm_sb = consts.tile([1, 8], F32)
        nc.vector.memset(warm_sb, 0.0)

        a2a_in = [dram.tile([2 * TP, HD, 512], BF16, name=f"a2a_in{h}",
                            tag=f"a2a_in{h}") for h in range(HPC)]
        a2a_out = [dram.tile([2 * TP, HD, 512], BF16, name=f"a2a_out{h}",
                             tag=f"a2a_out{h}") for h in range(HPC)]
        ssq_in = [dram.tile([1, 2, 512], F32, name=f"ssq_in{t}",
                            tag=f"ssq_in{t}") for t in range(4)]
        ssq_out = [dram.tile([1, 2, 512], F32, name=f"ssq_out{t}",
                             tag=f"ssq_out{t}") for t in range(4)]
        warm_in = dram.tile([1, 8], F32, name="warm_in", tag="warm_in")
        warm_out = dram.tile([1, 8], F32, name="warm_out", tag="warm_out")

        persist = outer.enter_context(tc.tile_pool(name="persist", bufs=1))
        qt_f = persist.tile([128, HPC, S], BF16)   # normed q
        kt_f = persist.tile([128, S], BF16)        # normed k
        v_sb = persist.tile([128, NT, HD], BF16)
        gt_sb = persist.tile([128, NH, 512], BF16)
        # attention exp tiles, alternated by head parity (sliced to 4m+4 blocks)
        et_bufs = [persist.tile([128, NT, 512], BF16, name=f"etb{i}")
                   for i in range(2)]

        # one PSUM pool spanning the whole interleaved program
        mm_cm = tc.tile_pool(name="mmps", bufs=1, space="PSUM")
        mmps = mm_cm.__enter__()

        def big_ps():
            return mmps.tile([128, 512], F32, name="big", tag="big", bufs=3)

        def v_ps(shape=None):
            return mmps.tile(shape or [128, 512], F32, name="vps", tag="vps",
                             bufs=2)

        def row_ps():
            return mmps.tile([1, 512], F32, name="rowp", tag="rowp", bufs=2)

        def bc_ps():
            return mmps.tile([128, 512], F32, name="bcp", tag="bcp", bufs=1)

        # pools that live through P1+P3
        p13 = ExitStack()
        rows = p13.enter_context(tc.tile_pool(name="rows", bufs=1))
        p1t = p13.enter_context(tc.tile_pool(name="p1t", bufs=2))
        proj = p13.enter_context(tc.tile_pool(name="proj", bufs=1))
        xtp = p13.enter_context(tc.tile_pool(name="xtp", bufs=1))
        smal = p13.enter_context(tc.tile_pool(name="smal", bufs=3))
        atp = p13.enter_context(tc.tile_pool(name="atp", bufs=1))
        p5t = p13.enter_context(tc.tile_pool(name="p5t", bufs=2))

        # ---- startup DMAs: warm the collective channel, then weights + x0
        nc.sync.dma_start(out=warm_in, in_=warm_sb)
        nc.gpsimd.collective_compute(
            "AllReduce", AluOp.add, replica_groups=groups4,
            ins=[warm_in.opt()], outs=[warm_out.opt()])

        wq_sb = proj.tile([128, ND, HPC * HD], BF16)
        wk_sb = proj.tile([128, ND, HD], BF16)
        wv_sb = proj.tile([128, ND, HD], BF16)
        # x chunks staged as two 8-tile halves, double-buffered
        xh0 = [xtp.tile([128, 8, 512], BF16, name=f"xts{i}", tag="xts",
                        bufs=2) for i in range(2)]
        for q4 in range(4):
            for dt in range(4 * q4, 4 * q4 + 4):
                rsl = slice(128 * dt, 128 * (dt + 1))
                nc.sync.dma_start(out=wq_sb[:, dt, :], in_=wqt_d[rsl, :])
            nc.sync.dma_start(
                out=xh0[q4 // 2][:, 4 * (q4 % 2):4 * (q4 % 2) + 4, :],
                in_=xt_d[512 * q4:512 * (q4 + 1), 0:512].rearrange(
                    "(n p) t -> p n t", p=128))
        nc.sync.dma_start(
            out=wk_sb, in_=wkt_d.rearrange("(n p) e -> p n e", p=128))
        nc.sync.dma_start(
            out=wv_sb, in_=wvt_d.rearrange("(n p) e -> p n e", p=128))
        nc.sync.dma_start(out=f12_sb, in_=f12_d[:, :, :])
        nc.sync.dma_start(out=utri_sb, in_=utri_d[:, :])
        nc.sync.dma_start(out=ltri_sb, in_=ltri_d[:, :])
        nc.sync.dma_start(out=wcol_sb, in_=wcol_d[:, :])
        nc.sync.dma_start(out=bsel_sb, in_=bsel_d[:, :])

        def load_xts(cols):
            halves = []
            for i in range(2):
                xh = xtp.tile([128, 8, 512], BF16, name=f"xts{i}", tag="xts",
                              bufs=2)
                nc.sync.dma_start(
                    out=xh,
                    in_=xt_d[1024 * i:1024 * (i + 1), cols].rearrange(
                        "(n p) t -> p n t", p=128))
                halves.append(xh)
            return [halves[dt // 8][:, dt % 8, :] for dt in range(ND)]

        def rope_emit(ps, dst, gcols):
            # dst = ev*F1 + swap64(ev)*F2, F tables indexed by position
            ev = p1t.tile([128, 512], F32, tag="ev")
            nc.scalar.copy(out=ev, in_=ps)
            evs = p1t.tile([128, 512], F32, tag="evs")
            nc.sync.dma_start(out=evs[0:64, :], in_=ev[64:128, :])
            nc.sync.dma_start(out=evs[64:128, :], in_=ev[0:64, :])
            m1 = p1t.tile([128, 512], F32, tag="m1")
            m2 = p1t.tile([128, 512], F32, tag="m2")
            nc.vector.tensor_mul(m1, ev, f12_sb[:, 0, gcols])
            nc.vector.tensor_mul(m2, evs, f12_sb[:, 1, gcols])
            nc.vector.tensor_tensor(out=dst, in0=m1, in1=m2, op=AluOp.add)

        def v_proj(t4, xtt):
            for tt in range(4):
                psv = v_ps([128, HD])
                for dt in range(ND):
                    nc.tensor.matmul(
                        psv, xtt[dt][:, 128 * tt:128 * (tt + 1)],
                        wv_sb[:, dt, :],
                        start=(dt == 0), stop=(dt == ND - 1))
                nc.scalar.copy(out=v_sb[:, 4 * t4 + tt, :], in_=psv)

        def ssq_ar(t4, qtn, ktn):
            sps = row_ps()
            for h in range(HPC):
                sq = p1t.tile([128, 512], BF16, tag="sq")
                nc.gpsimd.tensor_mul(sq, qtn[:, h, :], qtn[:, h, :])
                nc.tensor.matmul(sps, ones_colb, sq,
                                 start=(h == 0), stop=(h == HPC - 1))
            sq2 = rows.tile([1, 2, 512], F32, tag="sq2", name="sq2", bufs=2)
            nc.vector.tensor_copy(sq2[:, 0, :], sps)
            sps_k = row_ps()
            sqk = p1t.tile([128, 512], BF16, tag="sq")
            nc.gpsimd.tensor_mul(sqk, ktn, ktn)
            nc.tensor.matmul(sps_k, ones_colb, sqk, start=True, stop=True)
            nc.vector.tensor_copy(sq2[:, 1, :], sps_k)
            nc.sync.dma_start(out=ssq_in[t4][:, :, :], in_=sq2)
            nc.gpsimd.collective_compute(
                "AllReduce", AluOp.add, replica_groups=groups4,
                ins=[ssq_in[t4].opt()], outs=[ssq_out[t4].opt()])

        def proj_chunk(t4, xtt):
            cols = slice(512 * t4, 512 * (t4 + 1))
            qtn = p1t.tile([128, HPC, 512], BF16, tag="qtn", bufs=2)
            ktn = p1t.tile([128, 512], BF16, tag="ktn", bufs=2)
            for h in range(HPC):
                ps = big_ps()
                for dt in range(ND):
                    nc.tensor.matmul(
                        ps, wq_sb[:, dt, HD * h:HD * (h + 1)], xtt[dt],
                        start=(dt == 0), stop=(dt == ND - 1))
                rope_emit(ps, qtn[:, h, :], cols)
            ps = big_ps()
            for dt in range(ND):
                nc.tensor.matmul(ps, wk_sb[:, dt, :], xtt[dt],
                                 start=(dt == 0), stop=(dt == ND - 1))
            rope_emit(ps, ktn, cols)
            v_proj(t4, xtt)
            ssq_ar(t4, qtn, ktn)
            return qtn, ktn

        def chunk_norm(t4, qtn, ktn):
            # rr = exp(-0.5*ln(ssq/n + eps)); q gets gamma column too
            cols = slice(512 * t4, 512 * (t4 + 1))
            rs_sb = rows.tile([1, 2, 512], F32, tag="rs", name="rs", bufs=2)
            nc.sync.dma_start(out=rs_sb, in_=ssq_out[t4][:, :, :])
            if debug:
                nc.sync.dma_start(out=dbg["ssq"][:, cols],
                                  in_=rs_sb.rearrange("p r s -> (p r) s"))
            tmp2 = rows.tile([1, 2, 512], F32, tag="lg", name="lg", bufs=2)
            rr2 = rows.tile([1, 2, 512], F32, tag="rr", name="rr", bufs=2)
            nc.scalar.activation(out=tmp2[:, 0, :], in_=rs_sb[:, 0, :],
                                 func=AFT.Ln, scale=1.0 / (NH * HD),
                                 bias=eps_sb)
            nc.scalar.activation(out=tmp2[:, 1, :], in_=rs_sb[:, 1, :],
                                 func=AFT.Ln, scale=1.0 / (NKV * HD),
                                 bias=eps_sb)
            nc.scalar.activation(out=rr2[:, 0, :], in_=tmp2[:, 0, :],
                                 func=AFT.Exp, scale=-0.5)
            nc.scalar.activation(out=rr2[:, 1, :], in_=tmp2[:, 1, :],
                                 func=AFT.Exp, scale=-0.5)
            bq = bc_ps()
            nc.tensor.matmul(bq, ones_row, rr2[:, 0, :], start=True, stop=True)
            rq_bc = smal.tile([128, 512], F32, tag="bcast")
            nc.vector.tensor_copy(rq_bc, bq)
            bk = bc_ps()
            nc.tensor.matmul(bk, ones_row, rr2[:, 1, :], start=True, stop=True)
            rk_bc = smal.tile([128, 512], F32, tag="bcast")
            nc.vector.tensor_copy(rk_bc, bk)
            for h in range(HPC):
                nc.vector.scalar_tensor_tensor(
                    out=qt_f[:, h, cols], in0=qtn[:, h, :],
                    scalar=wcol_sb[:, h:h + 1], in1=rq_bc,
                    op0=AluOp.mult, op1=AluOp.mult)
            nc.vector.tensor_tensor(out=kt_f[:, cols], in0=ktn,
                                    in1=rk_bc, op=AluOp.mult)

        # ---------------- attention round (heads pipelined, reduce
        # matmuls interleaved into the next head's score stream) ----------
        def attn_reduce_step(pend, kb, nkb):
            h, et, dn, at_ps = pend
            m = nkb // 4 - 1
            o = kb - 4 * m
            w = 512 if o < 0 else 512 - 128 * o
            nc.tensor.matmul(dn[:, 512 - w:], ones_colb,
                             et[:, kb, 512 - w:],
                             start=(kb == 0), stop=(kb == nkb - 1))
            nc.tensor.matmul(at_ps[:, 512 - w:], v_sb[:, kb, :],
                             et[:, kb, 512 - w:],
                             start=(kb == 0), stop=(kb == nkb - 1))

        def attn_epilogue(pend, m):
            h, et, dn, at_ps = pend
            rln = rows.tile([1, 512], F32, tag="rln", name="rln", bufs=2)
            nc.scalar.activation(out=rln, in_=dn, func=AFT.Ln, scale=1.0)
            rd = rows.tile([1, 512], F32, tag="rd", name="rd", bufs=2)
            nc.scalar.activation(out=rd, in_=rln, func=AFT.Exp, scale=-1.0)
            bc = bc_ps()
            nc.tensor.matmul(bc, ones_row, rd, start=True, stop=True)
            rdb = smal.tile([128, 512], F32, tag="bcast")
            nc.vector.tensor_copy(rdb, bc)
            at0 = atp.tile([128, 512], BF16, tag="at0")
            at1 = atp.tile([128, 512], BF16, tag="at1")
            nc.vector.scalar_tensor_tensor(
                out=at0, in0=at_ps, scalar=bsel_sb[:, 0:1], in1=rdb,
                op0=AluOp.mult, op1=AluOp.mult)
            nc.vector.scalar_tensor_tensor(
                out=at1, in0=at_ps, scalar=bsel_sb[:, 1:2], in1=rdb,
                op0=AluOp.mult, op1=AluOp.mult)
            nc.sync.dma_start(out=a2a_in[h][m, :, :], in_=at0)
            nc.sync.dma_start(out=a2a_in[h][TP + m, :, :], in_=at1)
            if m == 3:
                nc.gpsimd.collective_compute(
                    "AllToAll", AluOp.bypass, replica_groups=groups8,
                    ins=[a2a_in[h].opt()], outs=[a2a_out[h].opt()])
                for i in range(TP):
                    sA = p5t.tile([128, 512], BF16, tag="sA")
                    sB = p5t.tile([128, 512], BF16, tag="sB")
                    nc.sync.dma_start(out=sA, in_=a2a_out[h][i, :, :])
                    nc.sync.dma_start(out=sB, in_=a2a_out[h][TP + i, :, :])
                    nc.gpsimd.tensor_tensor(out=gt_sb[:, 4 * i + h, :],
                                            in0=sA, in1=sB, op=AluOp.add)

        def attn_round(m):
            nkb = 4 * m + 4
            qc0 = 512 * m
            pend = None
            for h in range(HPC):
                et = et_bufs[h % 2]
                for kb in range(nkb):
                    o = kb - 4 * m
                    w = 512 if o < 0 else 512 - 128 * o
                    st = big_ps()
                    nc.tensor.matmul(
                        st[:, 512 - w:], kt_f[:, 128 * kb:128 * (kb + 1)],
                        qt_f[:, h, qc0 + 512 - w:qc0 + 512],
                        start=True, stop=(o < 0))
                    if o >= 0:
                        nc.tensor.matmul(
                            st[:, 512 - w:], ltri_sb, utri_sb[:, 0:w],
                            start=False, stop=True)
                    if pend is not None:
                        attn_reduce_step(pend, kb, nkb)
                    nc.scalar.activation(out=et[:, kb, 512 - w:],
                                         in_=st[:, 512 - w:],
                                         func=AFT.Exp, scale=SCALE)
                if pend is not None:
                    attn_epilogue(pend, m)
                pend = (h, et, row_ps(), v_ps())
            # drain the last head (no next score stream to interleave into)
            for kb in range(nkb):
                attn_reduce_step(pend, kb, nkb)
            attn_epilogue(pend, m)

        # ------------------------ interleaved schedule --------------------
        def wo_half_tiles(dc):
            xw = []
            for i in range(2):
                xh = xtp.tile([128, 8, 512], BF16, name=f"xts{i}", tag="xts",
                              bufs=2)
                nc.sync.dma_start(
                    out=xh,
                    in_=wot_d[1024 * i:1024 * (i + 1),
                              512 * dc:512 * (dc + 1)].rearrange(
                        "(n p) e -> p n e", p=128))
                xw.append(xh)
            return [xw[n // 8][:, n % 8, :] for n in range(NH)]

        xtt0 = [xh0[dt // 8][:, dt % 8, :] for dt in range(ND)]
        qk0 = proj_chunk(0, xtt0)
        qk1 = proj_chunk(1, load_xts(slice(512, 1024)))
        chunk_norm(0, *qk0)
        qk2 = proj_chunk(2, load_xts(slice(1024, 1536)))
        chunk_norm(1, *qk1)
        attn_round(0)
        qk3 = proj_chunk(3, load_xts(slice(1536, 2048)))
        # prefetch wo dc0 into wq_sb once the last projection released it
        nc.sync.dma_start(
            out=wq_sb,
            in_=wot_d[:, 0:512].rearrange("(n p) e -> p n e", p=128))
        chunk_norm(2, *qk2)
        attn_round(1)
        attn_round(2)
        chunk_norm(3, *qk3)
        wo1 = wo_half_tiles(1)     # prefetch wo dc1 during round 3
        attn_round(3)

        if debug:
            nc.sync.dma_start(out=dbg["qt"][:, :, :], in_=qt_f)
            nc.sync.dma_start(out=dbg["kt"][:, :], in_=kt_f)
            nc.sync.dma_start(out=dbg["v"][:, :, :], in_=v_sb)
            nc.sync.dma_start(out=dbg["at"][:, :, :], in_=gt_sb)

        mm_cm.__exit__(None, None, None)

        # ---------------- P5: output projection (wo pre-staged) -----------
        with tc.tile_pool(name="pop", bufs=2, space="PSUM") as pop:
            for dc in range(4):
                if dc == 0:
                    wo_view = [wq_sb[:, n, :] for n in range(NH)]
                elif dc == 1:
                    wo_view = wo1
                elif dc == 2:
                    nc.sync.dma_start(
                        out=wq_sb,
                        in_=wot_d[:, 1024:1536].rearrange(
                            "(n p) e -> p n e", p=128))
                    wo_view = [wq_sb[:, n, :] for n in range(NH)]
                else:
                    wo_view = wo_half_tiles(3)
                opsd = [pop.tile([128, 512], F32, name=f"ops{t}",
                                 tag=f"tt{t}", bufs=2) for t in range(4)]
                for idx, e16 in enumerate(
                        [4 * i + h for h in range(HPC) for i in range(TP)]):
                    for tt in range(4):
                        nc.tensor.matmul(
                            opsd[tt], gt_sb[:, e16, 128 * tt:128 * (tt + 1)],
                            wo_view[e16],
                            start=(idx == 0), stop=(idx == NH - 1))
                for tt in range(4):
                    osb = smal.tile([128, 512], F32, tag="bcast")
                    nc.vector.tensor_copy(osb, opsd[tt])
                    nc.sync.dma_start(
                        out=out_d[128 * tt:128 * (tt + 1),
                                  512 * dc:512 * (dc + 1)],
                        in_=osb)
        p13.close()

    _split_sync_waits(nc)
    return nc


# ------------------------------------------------------------- host side --
_PERM = np.concatenate([np.arange(0, HD, 2), np.arange(1, HD, 2)])


def _prep(inputs):
    x = np.asarray(inputs["x"], np.float32)
    fc = np.asarray(inputs["freq_cis"], np.float32)
    wq = np.asarray(inputs["wq"], np.float32)
    wk = np.asarray(inputs["wk"], np.float32)
    wv = np.asarray(inputs["wv"], np.float32)
    wo = np.asarray(inputs["wo"], np.float32)
    qnw = np.asarray(inputs["q_norm_w"], np.float32)
    knw = np.asarray(inputs["k_norm_w"], np.float32)

    wq_p = wq.reshape(NH, HD, DIM)[:, _PERM, :]
    wk_p = wk.reshape(NKV, HD, DIM)[:, _PERM, :]
    qnw_p = qnw.reshape(NH, HD)[:, _PERM]
    knw_p = knw.reshape(NKV, HD)[:, _PERM]

    xt = [np.ascontiguousarray(x[b].T).astype(BF) for b in range(B)]
    wqt = [np.ascontiguousarray(
        wq_p[4 * g:4 * (g + 1)].reshape(4 * HD, DIM).T).astype(BF)
        for g in range(TP)]
    wkt = [np.ascontiguousarray(wk_p[g].T).astype(BF) for g in range(TP)]
    wvt = [np.ascontiguousarray(wv[g * HD:(g + 1) * HD].T).astype(BF)
           for g in range(TP)]
    wot = np.ascontiguousarray(wo.T).astype(BF)

    # rope tables: F1 = [cos|cos], F2 = [-sin|sin] along the partition halves
    cos = fc[:, :, 0, 0].T          # [64, S]
    sin = fc[:, :, 1, 0].T          # [64, S]
    f1 = np.concatenate([cos, cos], axis=0)
    f2 = np.concatenate([-sin, sin], axis=0)
    f12 = np.ascontiguousarray(
        np.stack([f1, f2], axis=1)).astype(BF)            # [128, 2, S]

    wcol = []
    for g in range(TP):
        cols = np.empty((HD, HPC), np.float32)
        for hl in range(HPC):
            cols[:, hl] = qnw_p[4 * g + hl] * knw_p[g]
        wcol.append(np.ascontiguousarray(cols))

    j_idx = np.arange(128)[:, None]
    q_idx = np.arange(512)[None, :]
    utri = np.ascontiguousarray(
        np.where(j_idx > q_idx, -1000.0, 0.0).astype(BF))       # [128, 512]
    ltri = np.ascontiguousarray(np.eye(128, dtype=np.float32).astype(BF))

    in_maps = []
    for c in range(8):
        b, g = divmod(c, TP)
        bsel = np.zeros((128, 2), np.float32)
        bsel[:, b] = 1.0
        in_maps.append({
            "xt": xt[b], "wqt": wqt[g], "wkt": wkt[g], "wvt": wvt[g],
            "wot": wot, "f12": f12, "wcol": wcol[g], "utri": utri,
            "ltri": ltri, "bsel": bsel,
        })
    return in_maps


_GRAPH_CACHE = {}


def _get_graph(debug=False):
    key = bool(debug)
    if key not in _GRAPH_CACHE:
        _GRAPH_CACHE[key] = build_graph(debug=key)
    return _GRAPH_CACHE[key]


LAST_RESULT = None


def kernel(debug=False, _run_kwargs=None, **inputs):
    global LAST_RESULT
    from concourse.bass_utils import run_bass_kernel_spmd

    nc = _get_graph(debug=debug)
    in_maps = _prep(inputs)
    res = run_bass_kernel_spmd(nc, in_maps, core_ids=list(range(8)),
                               **(_run_kwargs or {}))
    LAST_RESULT = res
    out = np.empty((B, S, DIM), np.float32)
    for c in range(8):
        b, g = divmod(c, TP)
        out[b, TOK * g:TOK * (g + 1), :] = res.results[c]["out"]
    if debug:
        return out, res
    return out
